# revision 1
# baseline (speedup 1.0000x reference)
"""Trainium2 Bass kernel for 3x3 VALID conv (nn_BreakupConv).

x [16,64,128,128] f32, weights [128,64,9] f32 -> out [16,128,126,126] f32.

Strategy:
- Data-parallel: 2 images per NeuronCore (8 cores).
- Conv as 6 accumulating matmuls per 4-output-row tile: contraction over
  C_in (64) for one 3x3 tap per partition-half. Host prepares x in a
  [128, 16384] layout per image: partitions 0-63 = the image (channels
  0-63, flattened HxW), partitions 64-127 = the same image shifted by +1
  element (next column). A K=128 matmul covers two horizontally-adjacent
  taps at once; taps at kx=2 use a zero lower half in the weights.
- Matmul dtype float16 (host-cast): full-rate PE streaming (1 cycle/row vs 4
  for fp32) and FWL-eligible weight loads; measured output rel err ~2.7e-4
  vs the fp32 reference. CONV_MM_DTYPE=float32r gives ~1.4e-4 at ~1.25x the
  PE time; CONV_MM_DTYPE=float32 gives exact fp32 at ~4x.
"""

import os
import numpy as np

os.environ.setdefault("BASS_NEVER_TRACE", "1")

B, C_IN, H, W = 16, 64, 128, 128
C_OUT, HO, WO = 128, 126, 126
N_CORES = 8
IMGS_PER_CORE = B // N_CORES
HW = H * W           # 16384
ROWS_PER_TILE = 4    # output rows per PSUM tile (4*126 = 504 <= 512)

# matmul slots: (kind, rhs offset (ky,kx), upper tap, lower tap).
# "full": K=128 over both halves; the lower half holds x shifted by +1
# column, so its effective tap is (ky, kx+1). Taps at kx=2 get a zero
# lower-half weight block in the classic plan.
# The packed plan issues the kx=2 taps as K=64 matmuls instead; the
# adjacent upper/lower pair lands on disjoint PE row groups and can
# execute concurrently (row-tiling), saving ~1 matmul span per tile.
SLOTS_CLASSIC = [
    ("full", (0, 0), 0, 1),
    ("full", (1, 0), 3, 4),
    ("full", (2, 0), 6, 7),
    ("full", (0, 2), 2, None),
    ("full", (1, 2), 5, None),
    ("full", (2, 2), 8, None),
]
SLOTS_PACKED = [
    ("full", (0, 0), 0, 1),
    ("full", (1, 0), 3, 4),
    ("full", (2, 0), 6, 7),
    ("upper", (2, 2), 8, None),
    ("lower", (0, 1), None, 2),
    ("upper", (1, 2), 5, None),
]
SLOTS = (
    SLOTS_PACKED if os.environ.get("CONV_SLOTS", "classic") == "packed"
    else SLOTS_CLASSIC
)

MM_DTYPE = os.environ.get("CONV_MM_DTYPE", "float16")
# "host": send the shifted lower half from the host (2x input bytes).
# "pe": send only the 64 channels; generate the shifted lower half on-chip
#       with a K=64 identity matmul (cross-partition move via PSUM) + DVE
#       copy. Cuts HBM+port DMA traffic by ~18%.
DUP_MODE = os.environ.get("CONV_DUP", "host")
N_WSLOTS_EXTRA = 1  # identity block appended to wmm for DUP_MODE == "pe"

_CACHE = {}


N_CHUNKS = int(os.environ.get("CONV_CHUNKS", "16"))
OUT_PAIR = os.environ.get("CONV_OUT_PAIR", "0") == "1"
PSUM_BUFS = int(os.environ.get("CONV_PSUM_BUFS", "6"))
OUT_BUFS = int(os.environ.get("CONV_OUT_BUFS", "6"))


def _build_program(reps=1, mm_dtype=None, n_chunks=None, psum_bufs=None,
                   out_bufs=None, xp_bufs=2, hp_dma=False):
    import concourse.bacc as bacc
    import concourse.mybir as mybir
    from concourse.tile import TileContext

    dt = getattr(mybir.dt, mm_dtype or MM_DTYPE)
    f32 = mybir.dt.float32
    n_chunks = n_chunks or N_CHUNKS
    psum_bufs = psum_bufs or PSUM_BUFS
    out_bufs = out_bufs or OUT_BUFS

    pe_dup = DUP_MODE == "pe"
    sb_dup = DUP_MODE == "sb"
    n_wblk = len(SLOTS) + (N_WSLOTS_EXTRA if pe_dup else 0)
    x_parts = C_IN if (pe_dup or sb_dup) else 128
    nc = bacc.Bacc(None, target_bir_lowering=False)
    x2_d = nc.dram_tensor("x2", [IMGS_PER_CORE, x_parts, HW], dt,
                          kind="ExternalInput")
    w_d = nc.dram_tensor("wmm", [128, n_wblk * 128], dt, kind="ExternalInput")
    out_d = nc.dram_tensor("out2", [IMGS_PER_CORE, C_OUT, HO * WO], f32,
                           kind="ExternalOutput")

    n_tiles = (HO + ROWS_PER_TILE - 1) // ROWS_PER_TILE

    with TileContext(nc) as tc:
        with (
            tc.tile_pool(name="xp", bufs=xp_bufs) as xp,
            tc.tile_pool(name="wp", bufs=1) as wp,
            tc.tile_pool(name="pp", bufs=psum_bufs, space="PSUM") as pp,
            tc.tile_pool(name="op", bufs=out_bufs) as op,
        ):
            w_sb = wp.tile([128, n_wblk * 128], dt)
            nc.sync.dma_start(out=w_sb[:], in_=w_d[:])
            n_warm = int(os.environ.get("CONV_WARMUP", "0"))
            if n_warm:
                # PE warmup during the initial x DMA: ramps the clock gate
                # before real tiles start. Output is never read.
                warm_ps = pp.tile([128, 504], f32, name="warm_ps", tag="warm",
                                  bufs=1)
                for wi in range(n_warm):
                    nc.tensor.matmul(
                        warm_ps[:],
                        w_sb[:, 0:128],
                        w_sb[:, 0:504],
                        start=(wi == 0),
                        stop=(wi == n_warm - 1),
                    )
            for img in [i % IMGS_PER_CORE for i in range(reps * IMGS_PER_CORE)]:
                x_sb = xp.tile([128, HW], dt)
                csz = HW // n_chunks
                import contextlib
                hp_ctx = tc.high_priority() if hp_dma else contextlib.nullcontext()
                with hp_ctx:
                    for ci in range(n_chunks):
                        nc.sync.dma_start(
                            out=x_sb[0:x_parts, ci * csz:(ci + 1) * csz],
                            in_=x2_d[img, :, ci * csz:(ci + 1) * csz],
                        )
                if sb_dup:
                    # Shifted lower half via SBUF->SBUF DMA from the upper
                    # half: trades the 4.2 MiB HBM re-read for port traffic.
                    nc.any.memset(x_sb[C_IN:128, HW - 1:HW], 0.0)
                    for ci in range(n_chunks):
                        lo = ci * csz
                        n = csz if ci < n_chunks - 1 else csz - 1
                        nc.sync.dma_start(
                            out=x_sb[C_IN:128, lo:lo + n],
                            in_=x_sb[0:C_IN, lo + 1:lo + 1 + n],
                        )
                if pe_dup:
                    # Build the shifted lower half on-chip: identity matmul
                    # moves x[c, f+1] to partition 64+c via PSUM, DVE copies
                    # it back into SBUF. Saves the HBM re-read of the dup.
                    id_blk = w_sb[0:C_IN, len(SLOTS) * 128:n_wblk * 128]
                    nc.any.memset(x_sb[C_IN:128, HW - 1:HW], 0.0)
                    DB = 512
                    for b in range(HW // DB):
                        nb = DB if b < HW // DB - 1 else DB - 1
                        dup_ps = pp.tile([128, DB], f32, name="dup_ps",
                                         tag="dup", bufs=2)
                        nc.tensor.matmul(
                            dup_ps[:, 0:nb],
                            id_blk,
                            x_sb[0:C_IN, b * DB + 1:b * DB + 1 + nb],
                            start=True, stop=True,
                        )
                        nc.vector.tensor_copy(
                            x_sb[C_IN:128, b * DB:b * DB + nb],
                            dup_ps[C_IN:128, 0:nb],
                        )
                xv = x_sb[:].rearrange("p (h w) -> p h w", h=H)

                def do_tile(y, r, ot, oc):
                    ps = pp.tile([128, r * WO], f32, name="ps", tag="ps")
                    for s, (kind, (ky, kx), _ta, _tb) in enumerate(SLOTS):
                        if kind == "full":
                            psel = slice(0, 128)
                        elif kind == "upper":
                            psel = slice(0, 64)
                        else:
                            psel = slice(64, 128)
                        nc.tensor.matmul(
                            ps[:],
                            w_sb[psel, s * 128:(s + 1) * 128],
                            xv[psel, y + ky:y + ky + r, kx:kx + WO],
                            start=(s == 0),
                            stop=(s == len(SLOTS) - 1),
                        )
                    nc.vector.tensor_copy(ot[:, oc:oc + r * WO], ps[:])

                if OUT_PAIR:
                    group = 2 * ROWS_PER_TILE
                    for y0 in range(0, HO, group):
                        rg = min(group, HO - y0)
                        ot = op.tile([128, rg * WO], f32, name="ot", tag="ot")
                        for y in range(y0, min(y0 + group, HO), ROWS_PER_TILE):
                            r = min(ROWS_PER_TILE, HO - y)
                            do_tile(y, r, ot, (y - y0) * WO)
                        nc.sync.dma_start(
                            out=out_d[img, :, y0 * WO:(y0 + rg) * WO],
                            in_=ot[:],
                        )
                else:
                    for t in range(n_tiles):
                        y = ROWS_PER_TILE * t
                        r = min(ROWS_PER_TILE, HO - y)
                        ot = op.tile([128, r * WO], f32, name="ot", tag="ot")
                        do_tile(y, r, ot, 0)
                        nc.sync.dma_start(
                            out=out_d[img, :, y * WO:(y + r) * WO], in_=ot[:]
                        )
    nc.compile()
    return nc


def _build_nop_program():
    """Minimal program with the same I/O contract, for dispatch-floor timing."""
    import concourse.bacc as bacc
    import concourse.mybir as mybir
    from concourse.tile import TileContext

    f32 = mybir.dt.float32
    nc = bacc.Bacc(None, target_bir_lowering=False)
    x_d = nc.dram_tensor("xn", [128, 128], f32, kind="ExternalInput")
    o_d = nc.dram_tensor("on", [128, 128], f32, kind="ExternalOutput")
    with TileContext(nc) as tc:
        with tc.tile_pool(name="p", bufs=1) as p:
            t = p.tile([128, 128], f32)
            nc.sync.dma_start(out=t[:], in_=x_d[:])
            nc.sync.dma_start(out=o_d[:], in_=t[:])
    nc.compile()
    return nc


def _make_runner(nc):
    """Build a reusable jitted SPMD callable for `nc` over 8 cores.

    Returns (run, meta): run(list_of_global_np_inputs) -> list of global
    np outputs with shape (N_CORES*dim0, ...). Inputs are device_put once
    per call; no donation (outputs fully written by the kernel).
    """
    import jax
    import concourse.mybir as mybir
    from concourse import bass2jax
    from jax.experimental.shard_map import shard_map
    from jax.sharding import Mesh, NamedSharding, PartitionSpec

    bass2jax.install_neuronx_cc_hook()

    partition_name = (
        nc.partition_id_tensor.name if nc.partition_id_tensor is not None else None
    )
    in_names, out_names, out_avals, zero_outs = [], [], [], []
    for alloc in nc.m.functions[0].allocations:
        if not isinstance(alloc, mybir.MemoryLocationSet):
            continue
        name = alloc.memorylocations[0].name
        if alloc.kind == "ExternalInput":
            if name != partition_name:
                in_names.append(name)
        elif alloc.kind == "ExternalOutput":
            out_names.append(name)
            shape = tuple(alloc.tensor_shape)
            dtype = mybir.dt.np(alloc.dtype)
            out_avals.append(jax.core.ShapedArray(shape, dtype))
            zero_outs.append(np.zeros(shape, dtype))
    n_params = len(in_names)
    all_in_names = list(in_names) + list(out_names)
    if partition_name is not None:
        all_in_names.append(partition_name)

    def _body(*args):
        operands = list(args)
        if partition_name is not None:
            operands.append(bass2jax.partition_id_tensor())
        outs = bass2jax._bass_exec_p.bind(
            *operands,
            out_avals=tuple(out_avals),
            in_names=tuple(all_in_names),
            out_names=tuple(out_names),
            lowering_input_output_aliases=(),
            sim_require_finite=True,
            sim_require_nnan=True,
            nc=nc,
        )
        return tuple(outs)

    devices = jax.devices()[:N_CORES]
    mesh = Mesh(np.asarray(devices), ("core",))
    spec = PartitionSpec("core")
    n_args = n_params + len(out_names)
    sharded = jax.jit(
        shard_map(
            _body,
            mesh=mesh,
            in_specs=(spec,) * n_args,
            out_specs=(spec,) * len(out_names),
            check_rep=False,
        ),
        keep_unused=True,
    )
    sharding = NamedSharding(mesh, spec)
    zeros_dev = [
        jax.device_put(np.zeros((N_CORES * z.shape[0], *z.shape[1:]), z.dtype),
                       sharding)
        for z in zero_outs
    ]

    def run(global_inputs, device_inputs=None):
        if device_inputs is None:
            device_inputs = [jax.device_put(g, sharding) for g in global_inputs]
        outs = sharded(*device_inputs, *zeros_dev)
        jax.block_until_ready(outs)
        return outs

    meta = {
        "sharding": sharding,
        "out_avals": out_avals,
        "out_names": out_names,
        "jax": jax,
        "sharded": sharded,
        "zeros_dev": zeros_dev,
    }
    return run, meta


def get_runner(reps=1):
    key = ("runner", reps)
    if key not in _CACHE:
        nc = _build_program(reps)
        _CACHE[key] = _make_runner(nc)
    return _CACHE[key]


def get_nop_runner():
    if "nop_runner" not in _CACHE:
        nc = _build_nop_program()
        _CACHE["nop_runner"] = _make_runner(nc)
    return _CACHE["nop_runner"]


def _np_mm_dtype():
    return {"float16": np.float16, "bfloat16": None}.get(MM_DTYPE, np.float32)


def prep_inputs(x, weights):
    """Host-side shard prep: returns global (concat over cores) inputs."""
    npdt = _np_mm_dtype()
    if npdt is None:
        import ml_dtypes
        npdt = ml_dtypes.bfloat16
    x = np.asarray(x)
    weights = np.asarray(weights, dtype=np.float32).astype(npdt)

    base = np.asarray(x, dtype=np.float32).astype(npdt).reshape(B, C_IN, HW)
    if DUP_MODE in ("pe", "sb"):
        x2_global = base
    else:
        # [B, 128, HW]: upper 64 = channels, lower 64 = shifted +1 elem
        x2_global = np.empty((B, 2 * C_IN, HW), npdt)
        x2_global[:, :C_IN, :] = base
        x2_global[:, C_IN:, :HW - 1] = base[:, :, 1:]
        x2_global[:, C_IN:, HW - 1] = 0

    n_wblk = len(SLOTS) + (N_WSLOTS_EXTRA if DUP_MODE == "pe" else 0)
    wmm = np.zeros((128, n_wblk * 128), npdt)
    for s, (_kind, _off, ta, tb) in enumerate(SLOTS):
        if ta is not None:
            wmm[0:64, s * 128:(s + 1) * 128] = weights[:, :, ta].T
        if tb is not None:
            wmm[64:128, s * 128:(s + 1) * 128] = weights[:, :, tb].T
    if DUP_MODE == "pe":
        # identity block for on-chip dup: routes x[c] -> partition 64+c
        for c in range(C_IN):
            wmm[c, len(SLOTS) * 128 + C_IN + c] = 1.0
    wmm_global = np.tile(wmm, (N_CORES, 1))  # [8*128, 768]
    return [x2_global, wmm_global]


def kernel(x, weights):
    run, _meta = get_runner()
    outs = run(prep_inputs(x, weights))
    out_g = np.asarray(outs[0])  # [8*2, 128, HO*WO]
    return out_g.reshape(B, C_OUT, HO, WO)



# revision 2
# speedup vs baseline: 1.0703x; 1.0703x over previous
"""Trainium2 Bass kernel for 3x3 VALID conv (nn_BreakupConv).

x [16,64,128,128] f32, weights [128,64,9] f32 -> out [16,128,126,126] f32.

Scheme (v2, "image-split row tiling"):
- Data-parallel over 8 cores: 2 images per core.
- Per core, one SBUF x tile [128, 16384] fp16: partitions 0-63 hold image
  A's 64 channels, partitions 64-127 image B's. Every conv tap (ky,kx) is
  a K=64 matmul reading this UNSHIFTED buffer at offset (ky, kx): image A
  on PE row group (0,0), image B on (64,0). Consecutive A/B matmuls use
  disjoint PE row halves and accumulate into different PSUM banks, so the
  hardware runs them concurrently (row tiling) -> a tap pair streams in
  one 504-cycle span. 9 taps x 504 cycles covers TWO output tiles =
  the 4.5-span/128-row minimum for the 576-long contraction.
- PSUM banks drained to SBUF as fp16 by DVE (optionally split with ACT),
  DMA'd out in ~1 MiB groups.
- I/O per core: 4 MiB in + 8.1 MiB out (fp16) vs 8.4 + 16.3 MiB for v1.
"""

import os
import numpy as np

os.environ.setdefault("BASS_NEVER_TRACE", "1")

B, C_IN, H, W = 16, 64, 128, 128
C_OUT, HO, WO = 128, 126, 126
N_CORES = 8
IMGS_PER_CORE = B // N_CORES  # 2
HW = H * W                    # 16384
ROWS_PER_TILE = 4             # output rows per PSUM tile (4*126 = 504 <= 512)
TAPS = 9

MM_DTYPE = os.environ.get("CONV_MM_DTYPE", "float16")
OUT_DTYPE = os.environ.get("CONV_OUT_DT", "float16")
# "alt": per tile pair, taps interleaved A,B,A,B -> relies on the PE
#   background weight buffer to hide per-matmul LDWEIGHTS.
# "tapgroup": tap-outer over TAPGROUP tile pairs -> same-weight matmuls
#   run back-to-back so weight (re)loads amortize over the group.
ORDER = os.environ.get("CONV_ORDER", "alt")
TAPGROUP = int(os.environ.get("CONV_TAPGROUP", "3"))
N_CHUNKS = int(os.environ.get("CONV_CHUNKS", "8"))
G_TILES = int(os.environ.get("CONV_G", "8"))    # row tiles per out DMA
COPY_MODE = os.environ.get("CONV_COPY", "dve")  # "dve" | "split"
PS_BUFS = int(os.environ.get("CONV_PS_BUFS", "4"))   # PSUM bufs per image
OUT_BUFS = int(os.environ.get("CONV_OUT_BUFS", "2"))  # out bufs per image

_CACHE = {}


def _build_program(reps=1):
    import concourse.bacc as bacc
    import concourse.mybir as mybir
    from concourse.tile import TileContext

    dt = getattr(mybir.dt, MM_DTYPE)
    odt = getattr(mybir.dt, OUT_DTYPE)
    f32 = mybir.dt.float32

    nc = bacc.Bacc(None, target_bir_lowering=False)
    x_d = nc.dram_tensor("x2", [128, HW], dt, kind="ExternalInput")
    w_d = nc.dram_tensor("wmm", [128, TAPS * 128], dt, kind="ExternalInput")
    out_d = nc.dram_tensor("out2", [IMGS_PER_CORE, C_OUT, HO * WO], odt,
                           kind="ExternalOutput")

    with TileContext(nc) as tc:
        with (
            tc.tile_pool(name="xp", bufs=2) as xp,
            tc.tile_pool(name="wp", bufs=1) as wp,
            tc.tile_pool(name="pp", bufs=2 * PS_BUFS, space="PSUM") as pp,
            tc.tile_pool(name="op", bufs=2 * OUT_BUFS) as op,
        ):
            w_sb = wp.tile([128, TAPS * 128], dt)
            nc.sync.dma_start(out=w_sb[:], in_=w_d[:])

            def copy0(ot_slice, ps):
                nc.vector.tensor_copy(ot_slice, ps[:])

            def copy1(ot_slice, ps):
                if COPY_MODE == "split":
                    nc.scalar.copy(ot_slice, ps[:])
                else:
                    nc.vector.tensor_copy(ot_slice, ps[:])

            for _rep in range(reps):
                x_sb = xp.tile([128, HW], dt, name="x_sb", tag="x")
                csz = HW // N_CHUNKS
                for ci in range(N_CHUNKS):
                    nc.sync.dma_start(
                        out=x_sb[:, ci * csz:(ci + 1) * csz],
                        in_=x_d[:, ci * csz:(ci + 1) * csz])
                xv = x_sb[:].rearrange("p (h w) -> p h w", h=H)

                def mm(ps, img, t, y, r):
                    ky, kx = divmod(t, 3)
                    p0 = img * 64
                    nc.tensor.matmul(
                        ps[:],
                        w_sb[p0:p0 + 64, t * 128:(t + 1) * 128],
                        xv[p0:p0 + 64, y + ky:y + ky + r, kx:kx + WO],
                        start=(t == 0), stop=(t == TAPS - 1),
                    )

                for y0 in range(0, HO, G_TILES * ROWS_PER_TILE):
                    rg = min(G_TILES * ROWS_PER_TILE, HO - y0)
                    ots = [op.tile([128, rg * WO], odt, name=f"ot{i}",
                                   tag=f"ot{i}", bufs=OUT_BUFS)
                           for i in range(IMGS_PER_CORE)]
                    ys = list(range(y0, y0 + rg, ROWS_PER_TILE))
                    if ORDER == "alt":
                        for y in ys:
                            r = min(ROWS_PER_TILE, HO - y)
                            ps0 = pp.tile([128, r * WO], f32, name="ps0",
                                          tag="ps0", bufs=PS_BUFS)
                            ps1 = pp.tile([128, r * WO], f32, name="ps1",
                                          tag="ps1", bufs=PS_BUFS)
                            for t in range(TAPS):
                                mm(ps0, 0, t, y, r)
                                mm(ps1, 1, t, y, r)
                            oc = (y - y0) * WO
                            copy0(ots[0][:, oc:oc + r * WO], ps0)
                            copy1(ots[1][:, oc:oc + r * WO], ps1)
                    else:  # tapgroup
                        for gi in range(0, len(ys), TAPGROUP):
                            sub = ys[gi:gi + TAPGROUP]
                            pairs = []
                            for y in sub:
                                r = min(ROWS_PER_TILE, HO - y)
                                ps0 = pp.tile([128, r * WO], f32, name="ps0",
                                              tag="ps0", bufs=PS_BUFS)
                                ps1 = pp.tile([128, r * WO], f32, name="ps1",
                                              tag="ps1", bufs=PS_BUFS)
                                pairs.append((y, r, ps0, ps1))
                            for t in range(TAPS):
                                for (y, r, ps0, ps1) in pairs:
                                    mm(ps0, 0, t, y, r)
                                    mm(ps1, 1, t, y, r)
                            for (y, r, ps0, ps1) in pairs:
                                oc = (y - y0) * WO
                                copy0(ots[0][:, oc:oc + r * WO], ps0)
                                copy1(ots[1][:, oc:oc + r * WO], ps1)
                    for i in range(IMGS_PER_CORE):
                        nc.sync.dma_start(
                            out=out_d[i, :, y0 * WO:(y0 + rg) * WO],
                            in_=ots[i][:])
    nc.compile()
    return nc


def _build_nop_program():
    """Minimal program with the same I/O contract, for dispatch-floor timing."""
    import concourse.bacc as bacc
    import concourse.mybir as mybir
    from concourse.tile import TileContext

    f32 = mybir.dt.float32
    nc = bacc.Bacc(None, target_bir_lowering=False)
    x_d = nc.dram_tensor("xn", [128, 128], f32, kind="ExternalInput")
    o_d = nc.dram_tensor("on", [128, 128], f32, kind="ExternalOutput")
    with TileContext(nc) as tc:
        with tc.tile_pool(name="p", bufs=1) as p:
            t = p.tile([128, 128], f32)
            nc.sync.dma_start(out=t[:], in_=x_d[:])
            nc.sync.dma_start(out=o_d[:], in_=t[:])
    nc.compile()
    return nc


def _make_runner(nc):
    """Build a reusable jitted SPMD callable for `nc` over 8 cores.

    Returns (run, meta): run(list_of_global_np_inputs) -> list of global
    np outputs with shape (N_CORES*dim0, ...). Inputs are device_put once
    per call; no donation (outputs fully written by the kernel).
    """
    import jax
    import concourse.mybir as mybir
    from concourse import bass2jax
    from jax.experimental.shard_map import shard_map
    from jax.sharding import Mesh, NamedSharding, PartitionSpec

    bass2jax.install_neuronx_cc_hook()

    partition_name = (
        nc.partition_id_tensor.name if nc.partition_id_tensor is not None else None
    )
    in_names, out_names, out_avals, zero_outs = [], [], [], []
    for alloc in nc.m.functions[0].allocations:
        if not isinstance(alloc, mybir.MemoryLocationSet):
            continue
        name = alloc.memorylocations[0].name
        if alloc.kind == "ExternalInput":
            if name != partition_name:
                in_names.append(name)
        elif alloc.kind == "ExternalOutput":
            out_names.append(name)
            shape = tuple(alloc.tensor_shape)
            dtype = mybir.dt.np(alloc.dtype)
            out_avals.append(jax.core.ShapedArray(shape, dtype))
            zero_outs.append(np.zeros(shape, dtype))
    n_params = len(in_names)
    all_in_names = list(in_names) + list(out_names)
    if partition_name is not None:
        all_in_names.append(partition_name)

    def _body(*args):
        operands = list(args)
        if partition_name is not None:
            operands.append(bass2jax.partition_id_tensor())
        outs = bass2jax._bass_exec_p.bind(
            *operands,
            out_avals=tuple(out_avals),
            in_names=tuple(all_in_names),
            out_names=tuple(out_names),
            lowering_input_output_aliases=(),
            sim_require_finite=True,
            sim_require_nnan=True,
            nc=nc,
        )
        return tuple(outs)

    devices = jax.devices()[:N_CORES]
    mesh = Mesh(np.asarray(devices), ("core",))
    spec = PartitionSpec("core")
    n_args = n_params + len(out_names)
    sharded = jax.jit(
        shard_map(
            _body,
            mesh=mesh,
            in_specs=(spec,) * n_args,
            out_specs=(spec,) * len(out_names),
            check_rep=False,
        ),
        keep_unused=True,
    )
    sharding = NamedSharding(mesh, spec)
    zeros_dev = [
        jax.device_put(np.zeros((N_CORES * z.shape[0], *z.shape[1:]), z.dtype),
                       sharding)
        for z in zero_outs
    ]

    def run(global_inputs, device_inputs=None):
        if device_inputs is None:
            device_inputs = [jax.device_put(g, sharding) for g in global_inputs]
        outs = sharded(*device_inputs, *zeros_dev)
        jax.block_until_ready(outs)
        return outs

    meta = {
        "sharding": sharding,
        "out_avals": out_avals,
        "out_names": out_names,
        "jax": jax,
        "sharded": sharded,
        "zeros_dev": zeros_dev,
    }
    return run, meta


def get_runner(reps=1):
    key = ("runner", reps)
    if key not in _CACHE:
        nc = _build_program(reps)
        _CACHE[key] = _make_runner(nc)
    return _CACHE[key]


def get_nop_runner():
    if "nop_runner" not in _CACHE:
        nc = _build_nop_program()
        _CACHE["nop_runner"] = _make_runner(nc)
    return _CACHE["nop_runner"]


def _np_mm_dtype():
    return {"float16": np.float16, "bfloat16": None}.get(MM_DTYPE, np.float32)


def prep_inputs(x, weights):
    """Host-side shard prep: returns global (concat over cores) inputs."""
    npdt = _np_mm_dtype()
    if npdt is None:
        import ml_dtypes
        npdt = ml_dtypes.bfloat16
    x = np.asarray(x, dtype=np.float32).reshape(B, C_IN, HW).astype(npdt)
    # core c holds images 2c (partitions 0-63) and 2c+1 (64-127)
    x2_global = np.ascontiguousarray(
        x.reshape(N_CORES, IMGS_PER_CORE * C_IN, HW))

    w = np.asarray(weights, dtype=np.float32).astype(npdt)  # [128, 64, 9]
    wmm = np.zeros((128, TAPS * 128), npdt)
    for t in range(TAPS):
        wT = w[:, :, t].T  # [64, 128]
        wmm[0:64, t * 128:(t + 1) * 128] = wT
        wmm[64:128, t * 128:(t + 1) * 128] = wT
    wmm_global = np.tile(wmm, (N_CORES, 1))  # [8*128, 1152]
    return [x2_global.reshape(N_CORES * 2 * C_IN, HW), wmm_global]


def kernel(x, weights):
    run, _meta = get_runner()
    outs = run(prep_inputs(x, weights))
    out_g = np.asarray(outs[0])  # [16, 128, HO*WO] in OUT_DTYPE
    return out_g.reshape(B, C_OUT, HO, WO).astype(np.float32)


# revision 6
# speedup vs baseline: 1.4577x; 1.3619x over previous
"""Trainium2 Bass kernel for 3x3 VALID conv (nn_BreakupConv).

x [16,64,128,128] f32, weights [128,64,9] f32 -> out [16,128,126,126] f32.

Scheme (v2, "image-split row tiling"):
- Data-parallel over 8 cores: 2 images per core.
- Per core, one SBUF x tile [128, 16384] fp16: partitions 0-63 hold image
  A's 64 channels, partitions 64-127 image B's. Every conv tap (ky,kx) is
  a K=64 matmul reading this UNSHIFTED buffer at offset (ky, kx): image A
  on PE row group (0,0), image B on (64,0). Consecutive A/B matmuls use
  disjoint PE row halves and accumulate into different PSUM banks, so the
  hardware runs them concurrently (row tiling) -> a tap pair streams in
  one 504-cycle span. 9 taps x 504 cycles covers TWO output tiles =
  the 4.5-span/128-row minimum for the 576-long contraction.
- PSUM banks drained to SBUF as fp16 by DVE (optionally split with ACT),
  DMA'd out in ~1 MiB groups.
- I/O per core: 4 MiB in + 8.1 MiB out (fp16) vs 8.4 + 16.3 MiB for v1.
"""

import os
import numpy as np

os.environ.setdefault("BASS_NEVER_TRACE", "1")

B, C_IN, H, W = 16, 64, 128, 128
C_OUT, HO, WO = 128, 126, 126
N_CORES = 8
IMGS_PER_CORE = B // N_CORES  # 2
HW = H * W                    # 16384
ROWS_PER_TILE = 4             # output rows per PSUM tile (4*126 = 504 <= 512)
TAPS = 9

MM_DTYPE = os.environ.get("CONV_MM_DTYPE", "float16")
OUT_DTYPE = os.environ.get("CONV_OUT_DT", "float16")
# "alt": per tile pair, taps interleaved A,B,A,B -> relies on the PE
#   background weight buffer to hide per-matmul LDWEIGHTS.
# "tapgroup": tap-outer over TAPGROUP tile pairs -> same-weight matmuls
#   run back-to-back so weight (re)loads amortize over the group.
ORDER = os.environ.get("CONV_ORDER", "alt")
# "strided": matmul rhs is the 3D [64, r, 126] stride-128 window view.
# "contig": rhs is one contiguous span of (r-1)*128+126 elements covering
#   the window plus 6 junk columns (positionally consistent across taps);
#   the PSUM->SBUF copy drops the junk via a strided view. +1.2% PE cols,
#   but a pure 1D moving access pattern.
RHS_MODE = os.environ.get("CONV_RHS", "strided")
TAPGROUP = int(os.environ.get("CONV_TAPGROUP", "3"))
N_CHUNKS = int(os.environ.get("CONV_CHUNKS", "8"))
G_TILES = int(os.environ.get("CONV_G", "8"))    # row tiles per out DMA
COPY_MODE = os.environ.get("CONV_COPY", "dve")  # "dve" | "split"
PS_BUFS = int(os.environ.get("CONV_PS_BUFS", "4"))   # PSUM bufs per image
OUT_BUFS = int(os.environ.get("CONV_OUT_BUFS", "2"))  # out bufs per image

_CACHE = {}


def _build_program(reps=1):
    import concourse.bacc as bacc
    import concourse.mybir as mybir
    from concourse.tile import TileContext

    dt = getattr(mybir.dt, MM_DTYPE)
    odt = getattr(mybir.dt, OUT_DTYPE)
    f32 = mybir.dt.float32

    nc = bacc.Bacc(None, target_bir_lowering=False)
    x_d = nc.dram_tensor("x2", [128, HW], dt, kind="ExternalInput")
    w_d = nc.dram_tensor("wmm", [128, TAPS * 128], dt, kind="ExternalInput")
    out_d = nc.dram_tensor("out2", [IMGS_PER_CORE, C_OUT, HO * WO], odt,
                           kind="ExternalOutput")

    with TileContext(nc) as tc:
        with (
            tc.tile_pool(name="xp", bufs=2) as xp,
            tc.tile_pool(name="wp", bufs=1) as wp,
            tc.tile_pool(name="pp", bufs=2 * PS_BUFS, space="PSUM") as pp,
            tc.tile_pool(name="op", bufs=2 * OUT_BUFS) as op,
        ):
            w_sb = wp.tile([128, TAPS * 128], dt)
            nc.sync.dma_start(out=w_sb[:], in_=w_d[:])

            def copy0(ot_slice, ps_view):
                nc.vector.tensor_copy(ot_slice, ps_view)

            def copy1(ot_slice, ps_view):
                if COPY_MODE == "split":
                    nc.scalar.copy(ot_slice, ps_view)
                else:
                    nc.vector.tensor_copy(ot_slice, ps_view)

            for _rep in range(reps):
                x_sb = xp.tile([128, HW], dt, name="x_sb", tag="x")
                csz = HW // N_CHUNKS
                for ci in range(N_CHUNKS):
                    nc.sync.dma_start(
                        out=x_sb[:, ci * csz:(ci + 1) * csz],
                        in_=x_d[:, ci * csz:(ci + 1) * csz])
                xv = x_sb[:].rearrange("p (h w) -> p h w", h=H)

                def mm(ps, img, t, y, r):
                    ky, kx = divmod(t, 3)
                    p0 = img * 64
                    if RHS_MODE == "contig":
                        n = (r - 1) * W + WO
                        s = (y + ky) * W + kx
                        nc.tensor.matmul(
                            ps[:, 0:n],
                            w_sb[p0:p0 + 64, t * 128:(t + 1) * 128],
                            x_sb[p0:p0 + 64, s:s + n],
                            start=(t == 0), stop=(t == TAPS - 1),
                        )
                    else:
                        nc.tensor.matmul(
                            ps[:],
                            w_sb[p0:p0 + 64, t * 128:(t + 1) * 128],
                            xv[p0:p0 + 64, y + ky:y + ky + r, kx:kx + WO],
                            start=(t == 0), stop=(t == TAPS - 1),
                        )

                def ps_alloc(pool_tag, r, bufs):
                    if RHS_MODE == "contig":
                        t_ = pp.tile([128, 512], f32, name=pool_tag,
                                     tag=pool_tag, bufs=bufs)
                        return t_
                    return pp.tile([128, r * WO], f32, name=pool_tag,
                                   tag=pool_tag, bufs=bufs)

                def ps_read(ps, r):
                    if RHS_MODE == "contig":
                        return ps[:].rearrange(
                            "p (h w) -> p h w", w=W)[:, 0:r, 0:WO]
                    return ps[:]

                for y0 in range(0, HO, G_TILES * ROWS_PER_TILE):
                    rg = min(G_TILES * ROWS_PER_TILE, HO - y0)
                    ots = [op.tile([128, rg * WO], odt, name=f"ot{i}",
                                   tag=f"ot{i}", bufs=OUT_BUFS)
                           for i in range(IMGS_PER_CORE)]
                    ys = list(range(y0, y0 + rg, ROWS_PER_TILE))
                    if ORDER == "alt":
                        for y in ys:
                            r = min(ROWS_PER_TILE, HO - y)
                            ps0 = ps_alloc("ps0", r, PS_BUFS)
                            ps1 = ps_alloc("ps1", r, PS_BUFS)
                            for t in range(TAPS):
                                mm(ps0, 0, t, y, r)
                                mm(ps1, 1, t, y, r)
                            oc = (y - y0) * WO
                            copy0(ots[0][:, oc:oc + r * WO], ps_read(ps0, r))
                            copy1(ots[1][:, oc:oc + r * WO], ps_read(ps1, r))
                    else:  # tapgroup
                        for gi in range(0, len(ys), TAPGROUP):
                            sub = ys[gi:gi + TAPGROUP]
                            pairs = []
                            for y in sub:
                                r = min(ROWS_PER_TILE, HO - y)
                                ps0 = ps_alloc("ps0", r, PS_BUFS)
                                ps1 = ps_alloc("ps1", r, PS_BUFS)
                                pairs.append((y, r, ps0, ps1))
                            for t in range(TAPS):
                                for (y, r, ps0, ps1) in pairs:
                                    mm(ps0, 0, t, y, r)
                                    mm(ps1, 1, t, y, r)
                            for (y, r, ps0, ps1) in pairs:
                                oc = (y - y0) * WO
                                copy0(ots[0][:, oc:oc + r * WO],
                                      ps_read(ps0, r))
                                copy1(ots[1][:, oc:oc + r * WO],
                                      ps_read(ps1, r))
                    for i in range(IMGS_PER_CORE):
                        nc.sync.dma_start(
                            out=out_d[i, :, y0 * WO:(y0 + rg) * WO],
                            in_=ots[i][:])
    nc.compile()
    return nc


def _build_nop_program():
    """Minimal program with the same I/O contract, for dispatch-floor timing."""
    import concourse.bacc as bacc
    import concourse.mybir as mybir
    from concourse.tile import TileContext

    f32 = mybir.dt.float32
    nc = bacc.Bacc(None, target_bir_lowering=False)
    x_d = nc.dram_tensor("xn", [128, 128], f32, kind="ExternalInput")
    o_d = nc.dram_tensor("on", [128, 128], f32, kind="ExternalOutput")
    with TileContext(nc) as tc:
        with tc.tile_pool(name="p", bufs=1) as p:
            t = p.tile([128, 128], f32)
            nc.sync.dma_start(out=t[:], in_=x_d[:])
            nc.sync.dma_start(out=o_d[:], in_=t[:])
    nc.compile()
    return nc


def _make_runner(nc):
    """Build a reusable jitted SPMD callable for `nc` over 8 cores.

    Returns (run, meta): run(list_of_global_np_inputs) -> list of global
    np outputs with shape (N_CORES*dim0, ...). Inputs are device_put once
    per call; no donation (outputs fully written by the kernel).
    """
    import jax
    import concourse.mybir as mybir
    from concourse import bass2jax
    from jax.experimental.shard_map import shard_map
    from jax.sharding import Mesh, NamedSharding, PartitionSpec

    bass2jax.install_neuronx_cc_hook()

    partition_name = (
        nc.partition_id_tensor.name if nc.partition_id_tensor is not None else None
    )
    in_names, out_names, out_avals, zero_outs = [], [], [], []
    for alloc in nc.m.functions[0].allocations:
        if not isinstance(alloc, mybir.MemoryLocationSet):
            continue
        name = alloc.memorylocations[0].name
        if alloc.kind == "ExternalInput":
            if name != partition_name:
                in_names.append(name)
        elif alloc.kind == "ExternalOutput":
            out_names.append(name)
            shape = tuple(alloc.tensor_shape)
            dtype = mybir.dt.np(alloc.dtype)
            out_avals.append(jax.core.ShapedArray(shape, dtype))
            zero_outs.append(np.zeros(shape, dtype))
    n_params = len(in_names)
    all_in_names = list(in_names) + list(out_names)
    if partition_name is not None:
        all_in_names.append(partition_name)

    def _body(*args):
        operands = list(args)
        if partition_name is not None:
            operands.append(bass2jax.partition_id_tensor())
        outs = bass2jax._bass_exec_p.bind(
            *operands,
            out_avals=tuple(out_avals),
            in_names=tuple(all_in_names),
            out_names=tuple(out_names),
            lowering_input_output_aliases=(),
            sim_require_finite=True,
            sim_require_nnan=True,
            nc=nc,
        )
        return tuple(outs)

    devices = jax.devices()[:N_CORES]
    mesh = Mesh(np.asarray(devices), ("core",))
    spec = PartitionSpec("core")
    n_args = n_params + len(out_names)
    sharded = jax.jit(
        shard_map(
            _body,
            mesh=mesh,
            in_specs=(spec,) * n_args,
            out_specs=(spec,) * len(out_names),
            check_rep=False,
        ),
        keep_unused=True,
    )
    sharding = NamedSharding(mesh, spec)
    zeros_dev = [
        jax.device_put(np.zeros((N_CORES * z.shape[0], *z.shape[1:]), z.dtype),
                       sharding)
        for z in zero_outs
    ]

    def run(global_inputs, device_inputs=None):
        if device_inputs is None:
            device_inputs = [jax.device_put(g, sharding) for g in global_inputs]
        outs = sharded(*device_inputs, *zeros_dev)
        jax.block_until_ready(outs)
        return outs

    meta = {
        "sharding": sharding,
        "out_avals": out_avals,
        "out_names": out_names,
        "jax": jax,
        "sharded": sharded,
        "zeros_dev": zeros_dev,
    }
    return run, meta


def get_runner(reps=1):
    key = ("runner", reps)
    if key not in _CACHE:
        nc = _build_program(reps)
        _CACHE[key] = _make_runner(nc)
    return _CACHE[key]


def get_nop_runner():
    if "nop_runner" not in _CACHE:
        nc = _build_nop_program()
        _CACHE["nop_runner"] = _make_runner(nc)
    return _CACHE["nop_runner"]


def _np_mm_dtype():
    return {"float16": np.float16, "bfloat16": None}.get(MM_DTYPE, np.float32)


def prep_inputs(x, weights):
    """Host-side shard prep: returns global (concat over cores) inputs."""
    npdt = _np_mm_dtype()
    if npdt is None:
        import ml_dtypes
        npdt = ml_dtypes.bfloat16
    x = np.asarray(x, dtype=np.float32).reshape(B, C_IN, HW).astype(npdt)
    # core c holds images 2c (partitions 0-63) and 2c+1 (64-127)
    x2_global = np.ascontiguousarray(
        x.reshape(N_CORES, IMGS_PER_CORE * C_IN, HW))

    w = np.asarray(weights, dtype=np.float32).astype(npdt)  # [128, 64, 9]
    wmm = np.zeros((128, TAPS * 128), npdt)
    for t in range(TAPS):
        wT = w[:, :, t].T  # [64, 128]
        wmm[0:64, t * 128:(t + 1) * 128] = wT
        wmm[64:128, t * 128:(t + 1) * 128] = wT
    wmm_global = np.tile(wmm, (N_CORES, 1))  # [8*128, 1152]
    return [x2_global.reshape(N_CORES * 2 * C_IN, HW), wmm_global]


def kernel(x, weights):
    run, _meta = get_runner()
    outs = run(prep_inputs(x, weights))
    out_g = np.asarray(outs[0])  # [16, 128, HO*WO] in OUT_DTYPE
    return out_g.reshape(B, C_OUT, HO, WO).astype(np.float32)
